# revision 17
# baseline (speedup 1.0000x reference)
"""Sparse (masked) multi-head attention on 8 Trainium2 NeuronCores.

Problem: nodes [2,2048,512], edge_mask [2,2048,2048] (bool),
q/kv/o linear layers with H=8 heads of DH=64.

Sharding: batch x head-group.  Core c handles batch b = c//4 and head group
g = c%4 (heads 2g, 2g+1 = inner columns g*128:(g+1)*128).  Each core
computes its two heads' attention over the full sequence plus its partial
contribution to the output projection; the host sums the 4 partials per
batch and adds bo (plus the constant bv @ wo term).

v4 design: all matmuls bf16 (fp8 on the data path costs ~3% output error
each - over the tolerance).  The exp+mask work (the baseline's pacing
bottleneck at ~1 us/tile on ScalarE) is split across three engines by
head so the PE's 68 us of matmul work becomes the only critical path:
  h=1: ScalarE true exp -> bf16 P; mask via GpSimd multiply (j-blocks
       0-11, fp8 {0,1} mask operand) or VectorE multiply (j-blocks 12-15).
  h=0: VectorE scalar_tensor_tensor computes Schraudolph exp fused with
       the mask: int16 bits = round(sim*23.083 + (16384*m + 512)),
       reinterpreted as bf16.  Masked entries land at tiny-positive bf16
       (~1e-37); unmasked P carries a constant 2^5 factor that cancels in
       the softmax normalization.
Sim matmuls run K=64 (no zero-padding; cost is free-size-only).  AV
accumulates [v|1] @ P^T per j-block (denominator via the ones column).
AV emission lags two j-superblocks behind sim/exp so the PE never waits
on the exp engines.  The output projection PSUM DMAs straight to HBM as
f32, interleaved with the second i-half's attention.
"""
import numpy as np
import ml_dtypes

import concourse.bass as bass
import concourse.bacc as bacc
import concourse.tile as tile
from concourse import mybir
from concourse.bass_utils import run_bass_kernel_spmd
from bass_rust import add_dep_helper

B, N, DIM = 2, 2048, 512
H, DH = 8, 64
INNER = H * DH
SCALE = DH ** -0.5
NCORES = 8
HG = 128            # inner columns per core (2 heads x 64)
NJB = 16            # 128-row j blocks
NSB = 8             # 256-row j superblocks (pipeline unit)
NH = N // 2         # i-half size
NGJB = 12           # j-blocks 0-11: h=1 mask on GpSimd; 12-15: PE fold

BF16 = mybir.dt.bfloat16
F32 = mybir.dt.float32
FP8 = mybir.dt.float8e4
I16 = mybir.dt.int16
ts = bass.ts
ds = bass.ds

# Schraudolph: bits = sim*A + (16384*m + 512); bf16 exponent grid 128/ln2
A_SCHRAU = (128.0 / np.log(2.0)) * SCALE


def _build():
    nc = bacc.Bacc(monotonic_sem_count=0)
    n_d = nc.declare_dram_parameter("nodes16", [DIM, N], BF16, isOutput=False)
    wq_d = nc.declare_dram_parameter("wq16", [DIM, HG], BF16, isOutput=False)
    wk_d = nc.declare_dram_parameter("wk16", [DIM, HG], BF16, isOutput=False)
    wv_d = nc.declare_dram_parameter("wv16", [DIM, HG], BF16, isOutput=False)
    wo_d = nc.declare_dram_parameter("wo16", [HG, DIM], BF16, isOutput=False)
    bq_d = nc.declare_dram_parameter("bq1", [HG, 1], F32, isOutput=False)
    bk_d = nc.declare_dram_parameter("bk1", [HG, 1], F32, isOutput=False)
    mb_d = nc.declare_dram_parameter("mbias", [N, N], BF16, isOutput=False)
    m8_d = nc.declare_dram_parameter("mask8", [6 * 128, N], FP8, isOutput=False)
    m16_d = nc.declare_dram_parameter("mask16", [6 * 128, N], BF16, isOutput=False)
    fw_d = nc.declare_dram_parameter("foldW", [128, 128], BF16, isOutput=False)
    out_d = nc.declare_dram_parameter("out", [N, DIM], BF16, isOutput=True)

    with tile.TileContext(nc) as tc:
        with (
            tc.tile_pool(name="persist", bufs=1) as persist,
            tc.tile_pool(name="pth1", bufs=5) as pth1_p,
            tc.tile_pool(name="ptm", bufs=5) as ptm_p,
            tc.tile_pool(name="pt16", bufs=6) as pt16_p,
            tc.tile_pool(name="denp", bufs=1) as denp,
            tc.tile_pool(name="osbp", bufs=2) as osbp,
            tc.tile_pool(name="psA", bufs=1, space="PSUM") as psA,
            tc.tile_pool(name="psB", bufs=1, space="PSUM") as psB,
        ):
            # ---- loads: nodes first (projections gate on them) ----
            n16 = persist.tile([128, 4, N], BF16)
            n_r = n_d.rearrange("(c p) n -> p c n", p=128)
            nt_dmas = []
            for c in range(4):
                eng = [nc.scalar, nc.sync, nc.gpsimd, nc.scalar][c]
                d = eng.dma_start(out=n16[:, c, :], in_=n_r[:, c, :])
                nt_dmas.append(d)
            wq16 = persist.tile([128, 4, HG], BF16)
            nc.scalar.dma_start(out=wq16[:], in_=wq_d.rearrange("(c p) m -> p c m", p=128))
            wk16 = persist.tile([128, 4, HG], BF16)
            nc.scalar.dma_start(out=wk16[:], in_=wk_d.rearrange("(c p) m -> p c m", p=128))
            wv16 = persist.tile([128, 4, HG], BF16)
            nc.scalar.dma_start(out=wv16[:], in_=wv_d.rearrange("(c p) m -> p c m", p=128))
            wo16 = persist.tile([HG, DIM], BF16)
            nc.scalar.dma_start(out=wo16[:], in_=wo_d[:])
            bq1 = persist.tile([HG, 1], F32)
            nc.scalar.dma_start(out=bq1[:], in_=bq_d[:])
            bk1 = persist.tile([HG, 1], F32)
            nc.scalar.dma_start(out=bk1[:], in_=bk_d[:])
            foldW = persist.tile([128, 128], BF16)
            nc.scalar.dma_start(out=foldW[:], in_=fw_d[:])

            # masks after the projection-critical loads, in j order so the
            # first attention units' tiles land first
            mbias = persist.tile([128, NJB, N], BF16)
            mb_r = mb_d.rearrange("(g p) i -> p g i", p=128)
            mask8 = persist.tile([128, 6, N], FP8)
            m8_r = m8_d.rearrange("(g p) i -> p g i", p=128)
            mask16 = persist.tile([128, 6, N], BF16)
            m16_r = m16_d.rearrange("(g p) i -> p g i", p=128)
            mask_dmas = []
            for grp in range(4):
                d = nc.sync.dma_start(
                    out=mbias[:, ts(grp, 4), :], in_=mb_r[:, ts(grp, 4), :])
                mask_dmas.append(d)
                if grp < 3:
                    d = nc.sync.dma_start(
                        out=mask8[:, ts(grp, 2), :], in_=m8_r[:, ts(grp, 2), :])
                    mask_dmas.append(d)
                    d = nc.sync.dma_start(
                        out=mask16[:, ts(grp, 2), :], in_=m16_r[:, ts(grp, 2), :])
                    mask_dmas.append(d)
            for d in mask_dmas:
                add_dep_helper(d.ins, nt_dmas[-1].ins, reason="mask DMA after nodes")

            # ---- PE warm-up (clock ramp) while DMA streams ----
            wrm = persist.tile([128, 512], BF16)
            nc.vector.memset(wrm[:], 0.0)
            wps = psA.tile([128, 512], F32, tag="sim0")
            for i in range(10):
                nc.tensor.matmul(wps[:], lhsT=wrm[:, 0:128], rhs=wrm[:],
                                 start=(i == 0), stop=(i == 9))
            wout = persist.tile([128, 512], BF16)
            nc.vector.tensor_copy(wout[:], wps[:])

            # ---- q/k projections ----
            qT16 = persist.tile([128, N], BF16)
            kT16 = persist.tile([128, N], BF16)
            for half in range(2):
                qps = psA.tile([128, NH], F32, tag="sim0")
                kps = psA.tile([128, NH], F32, tag="sim1")
                for isl in range(2):
                    for c in range(4):
                        nc.tensor.matmul(
                            qps[:, ts(isl, 512)],
                            lhsT=wq16[:, c, :],
                            rhs=n16[:, c, ds(half * NH + isl * 512, 512)],
                            start=(c == 0), stop=(c == 3))
                for isl in range(2):
                    for c in range(4):
                        nc.tensor.matmul(
                            kps[:, ts(isl, 512)],
                            lhsT=wk16[:, c, :],
                            rhs=n16[:, c, ds(half * NH + isl * 512, 512)],
                            start=(c == 0), stop=(c == 3))
                nc.scalar.activation(
                    out=qT16[:, ts(half, NH)], in_=qps[:],
                    func=mybir.ActivationFunctionType.Identity, bias=bq1[:])
                nc.vector.tensor_scalar(
                    out=kT16[:, ts(half, NH)], in0=kps[:], scalar1=bk1[:],
                    scalar2=None, op0=mybir.AluOpType.add)

            # ---- v projection; v16[p, jb, 65h:65h+64] = v rows, col
            # 65h+64 = 1.0 (softmax denominator column)
            v16 = persist.tile([128, NJB, 130], BF16)
            nc.vector.memset(v16[:, :, 64:65], 1.0)
            nc.vector.memset(v16[:, :, 129:130], 1.0)
            for q4 in range(4):  # 4 j-blocks per psum fill
                vps = psB.tile([128, 4, HG], F32, tag="num0")
                for k4 in range(4):
                    jb = q4 * 4 + k4
                    for c in range(4):
                        nc.tensor.matmul(
                            vps[:, k4, :],
                            lhsT=n16[:, c, ts(jb, 128)],
                            rhs=wv16[:, c, :],
                            start=(c == 0), stop=(c == 3))
                dst = v16[:, ds(q4 * 4, 4), :].rearrange(
                    "p b (h c) -> p b h c", h=2)[:, :, :, 0:64]
                src = vps[:].rearrange("p b (h c) -> p b h c", h=2)
                if q4 % 2 == 0:
                    nc.scalar.copy(out=dst, in_=src)
                else:
                    nc.vector.tensor_copy(dst, src)

            # ---- attention ----
            attnT = persist.tile([128, N], BF16)
            ebias = persist.tile([128, 1], F32)
            nc.vector.memset(ebias[:], -99.0)
            out_r = out_d.rearrange("(g p) m -> p g m", p=128)
            nums = {}

            def sims_unit(ih, jsb, jb2):
                """sim + exp/stt for j-block jb=2*jsb+jb2, both heads."""
                io = ih * NH
                jb = 2 * jsb + jb2
                # h=1 scalar head: true exp; mask via PE-fold (jb%4==3,
                # spreads GpSimd load) or GpSimd multiply
                fold = jb >= NGJB
                sps = psA.tile([128, NH], F32, tag="sim1")
                for isl in range(2):
                    nc.tensor.matmul(
                        sps[:, ts(isl, 512)],
                        lhsT=kT16[64:128, ts(jb, 128)],
                        rhs=qT16[64:128, ds(io + isl * 512, 512)],
                        start=True, stop=not fold)
                if fold:
                    for isl in range(2):
                        nc.tensor.matmul(
                            sps[:, ts(isl, 512)],
                            lhsT=foldW[:],
                            rhs=mbias[:, jb, ds(io + isl * 512, 512)],
                            start=False, stop=True, skip_group_check=True)
                    ptm = ptm_p.tile([128, NH], BF16, tag="ptm")
                    nc.scalar.activation(
                        out=ptm[:], in_=sps[:],
                        func=mybir.ActivationFunctionType.Exp, scale=SCALE,
                        bias=ebias[:])
                elif jb % 2 == 0:
                    pth1 = pth1_p.tile([128, NH], BF16, tag="pth1")
                    nc.scalar.activation(
                        out=pth1[:], in_=sps[:],
                        func=mybir.ActivationFunctionType.Exp, scale=SCALE)
                    ptm = ptm_p.tile([128, NH], BF16, tag="ptm")
                    nc.gpsimd.tensor_mul(
                        ptm[:], pth1[:], mask8[:, jb // 2, ds(io, NH)])
                else:
                    ptm = ptm_p.tile([128, NH], BF16, tag="ptm")
                    nc.scalar.activation(
                        out=ptm[:], in_=sps[:],
                        func=mybir.ActivationFunctionType.Exp, scale=SCALE)
                    nc.vector.tensor_mul(
                        ptm[:], ptm[:], mask16[:, jb // 2, ds(io, NH)])
                # h=0 vector head: schraudolph exp fused with mask
                vps_ = psA.tile([128, NH], F32, tag="sim0")
                for isl in range(2):
                    nc.tensor.matmul(
                        vps_[:, ts(isl, 512)],
                        lhsT=kT16[0:64, ts(jb, 128)],
                        rhs=qT16[0:64, ds(io + isl * 512, 512)],
                        start=True, stop=True)
                pt16 = pt16_p.tile([128, NH], I16, tag="pt16")
                nc.vector.scalar_tensor_tensor(
                    out=pt16[:], in0=vps_[:], scalar=A_SCHRAU,
                    in1=mbias[:, jb, ds(io, NH)],
                    op0=mybir.AluOpType.mult, op1=mybir.AluOpType.add)
                return ptm, pt16

            def av_unit(ih, jsb, tiles):
                for jb2 in range(2):
                    jb = 2 * jsb + jb2
                    ptm, pt16 = tiles[jb2]
                    for isl in range(2):
                        nc.tensor.matmul(
                            nums[ih, 1][:, ts(isl, 512)],
                            lhsT=v16[:, jb, 65:130],
                            rhs=ptm[:, ts(isl, 512)],
                            start=(jsb == 0 and jb2 == 0),
                            stop=(jsb == NSB - 1 and jb2 == 1))
                    for isl in range(2):
                        nc.tensor.matmul(
                            nums[ih, 0][:, ts(isl, 512)],
                            lhsT=v16[:, jb, 0:65],
                            rhs=pt16[:].bitcast(BF16)[:, ts(isl, 512)],
                            start=(jsb == 0 and jb2 == 0),
                            stop=(jsb == NSB - 1 and jb2 == 1))

            def copy_nums(ih):
                # evacuate the AV accumulators to SBUF right away: frees
                # the num PSUM tags for the next i-half (the only cross-
                # half dependency) and lets the normalize chain run fully
                # off the critical path
                nsb0 = denp.tile([65, NH], BF16, tag="nsb0", name=f"nsbA{ih}")
                nc.scalar.copy(out=nsb0[:], in_=nums[ih, 0][:])
                nsb1 = denp.tile([65, NH], BF16, tag="nsb1", name=f"nsbB{ih}")
                nc.vector.tensor_copy(nsb1[:], nums[ih, 1][:])
                return [nsb0, nsb1]

            def rec_chain(ih, nsbs):
                recs = []
                for h in range(2):
                    den1 = denp.tile([1, NH], F32, tag=f"den1{h}", name=f"den1{ih}{h}")
                    nc.vector.tensor_copy(den1[:], nsbs[h][64:65, :])
                    rec1 = denp.tile([1, NH], F32, tag=f"rec1{h}", name=f"rec1{ih}{h}")
                    nc.vector.reciprocal_approx_fast(out=rec1[:], in_=den1[:])
                    rec = denp.tile([64, NH], F32, tag=f"rec{h}", name=f"rec{ih}{h}")
                    nc.gpsimd.partition_broadcast(rec[:], rec1[:])
                    recs.append(rec)
                return recs

            def attn_muls(ih, nsbs, recs):
                io = ih * NH
                for h in range(2):
                    nc.vector.tensor_mul(
                        attnT[ts(h, 64), ds(io, NH)], nsbs[h][0:64, :],
                        recs[h][:])

            def oproj_group(grp):
                tag = ["sim0", "sim1"][grp % 2]
                ops = psA.tile([128, 2, DIM], F32, tag=tag, name=f"ops{grp}")
                for k in range(2):
                    ib = grp * 2 + k
                    nc.tensor.matmul(
                        ops[:, k, :], lhsT=attnT[:, ts(ib, 128)], rhs=wo16[:],
                        start=True, stop=True)
                osb = osbp.tile([128, 2, DIM], BF16, tag="osb", name=f"osb{grp}")
                if grp % 2 == 0:
                    nc.scalar.copy(out=osb[:], in_=ops[:])
                else:
                    nc.vector.tensor_copy(osb[:], ops[:])
                eng = [nc.sync, nc.scalar][grp % 2]
                eng.dma_start(out=out_r[:, ds(grp * 2, 2), :], in_=osb[:])

            units = [(ih, jsb) for ih in range(2) for jsb in range(NSB)]
            pend = []
            nsbs0 = None
            for k, (ih, jsb) in enumerate(units):
                if k == 8:
                    # boundary: drain ihalf0's last AVs (their P is fold/
                    # GpSimd-free by construction), then evacuate the num
                    # accumulators so ihalf1's AV can claim the PSUM tags;
                    # the copies run while the PE drains
                    while pend:
                        av_unit(*pend.pop(0))
                    nsbs0 = copy_nums(0)
                if (ih, 0) not in nums:
                    nums[ih, 0] = psB.tile([65, NH], F32, tag="num0", name=f"numA{ih}")
                    nums[ih, 1] = psB.tile([65, NH], F32, tag="num1", name=f"numB{ih}")
                r0 = sims_unit(ih, jsb, 0)
                if len(pend) >= 2:
                    av_unit(*pend.pop(0))
                r1 = sims_unit(ih, jsb, 1)
                pend.append((ih, jsb, [r0, r1]))
            while pend:
                av_unit(*pend.pop(0))
            nsbs1 = copy_nums(1)
            recs0 = rec_chain(0, nsbs0)
            recs1 = rec_chain(1, nsbs1)
            attn_muls(0, nsbs0, recs0)
            for grp in range(4):
                oproj_group(grp)
            attn_muls(1, nsbs1, recs1)
            for grp in range(4, 8):
                oproj_group(grp)

    nc.compile()
    from collections import defaultdict
    next_id = defaultdict(lambda: 8)
    for a in nc.m.functions[0].allocations:
        if type(a).__name__ == "Register" and a.reg_id == -1:
            a.reg_id = next_id[str(a.engine)]
            next_id[str(a.engine)] += 1
    return nc


_NC_CACHE = None


def _get_nc():
    global _NC_CACHE
    if _NC_CACHE is None:
        _NC_CACHE = _build()
    return _NC_CACHE


def _prep_in_maps(nodes, edge_mask, wq, bq, wkv, bkv, wo, bo):
    f8 = ml_dtypes.float8_e4m3
    bf16 = ml_dtypes.bfloat16
    wk_full, wv_full = wkv[:, :INNER], wkv[:, INNER:]
    bk_full = bkv[:INNER]
    foldW = (np.eye(128, dtype=np.float32) * 0.046875).astype(bf16)
    per_batch = []
    for b in range(B):
        maskT = edge_mask[b].T
        mbias = (16384.0 * maskT + 512.0).astype(bf16)
        mT3 = maskT.reshape(NJB, 128, N)
        mask8 = mT3[0:NGJB:2].reshape(6 * 128, N).astype(np.float32).astype(f8)
        mask16 = mT3[1:NGJB:2].reshape(6 * 128, N).astype(np.float32).astype(bf16)
        nodes16 = np.ascontiguousarray(nodes[b].T).astype(bf16)
        per_batch.append((nodes16, mbias, mask8, mask16))
    in_maps = []
    for core in range(NCORES):
        b, g = core // 4, core % 4
        cs = slice(g * HG, (g + 1) * HG)
        nodes16, mbias, mask8, mask16 = per_batch[b]
        in_maps.append({
            "nodes16": nodes16,
            "wq16": np.ascontiguousarray(wq[:, cs]).astype(bf16),
            "wk16": np.ascontiguousarray(wk_full[:, cs]).astype(bf16),
            "wv16": np.ascontiguousarray(wv_full[:, cs]).astype(bf16),
            "wo16": np.ascontiguousarray(wo[cs, :]).astype(bf16),
            "bq1": np.ascontiguousarray(bq[cs]).reshape(HG, 1).astype(np.float32),
            "bk1": np.ascontiguousarray(bk_full[cs]).reshape(HG, 1).astype(np.float32),
            "mbias": mbias,
            "mask8": mask8,
            "mask16": mask16,
            "foldW": foldW,
        })
    return in_maps


def kernel(nodes, edge_mask, wq, bq, wkv, bkv, wo, bo, _trace=False, _trace_kwargs=None):
    nodes = np.asarray(nodes, dtype=np.float32)
    edge_mask = np.asarray(edge_mask)
    wq = np.asarray(wq, dtype=np.float32)
    bq = np.asarray(bq, dtype=np.float32)
    wkv = np.asarray(wkv, dtype=np.float32)
    bkv = np.asarray(bkv, dtype=np.float32)
    wo = np.asarray(wo, dtype=np.float32)
    bo = np.asarray(bo, dtype=np.float32)

    nc = _get_nc()
    in_maps = _prep_in_maps(nodes, edge_mask, wq, bq, wkv, bkv, wo, bo)
    kw = {}
    if _trace:
        kw = dict(trace=True, **(_trace_kwargs or {}))
    res = run_bass_kernel_spmd(nc, in_maps, list(range(NCORES)), **kw)
    out = np.zeros((B, N, DIM), np.float32)
    for core in range(NCORES):
        out[core // 4] += res.results[core]["out"].astype(np.float32)
    # v-bias shifts each head's attention output by exactly bv (softmax
    # weights sum to 1), so its output contribution is the constant bv @ wo.
    bv_full = bkv[INNER:]
    out += (bv_full @ wo + bo)[None, None, :]
    if _trace:
        return out, res
    return out
